# revision 38
# baseline (speedup 1.0000x reference)
"""Trainium2 Bass kernel for a binarized BasicBlock (2x bconv3x3 + BN +
residual hardtanh + channel shuffle), data-parallel over batch on 8 cores.

Self-contained: hardcodes shapes from the problem spec.
  x: (32, 256, 56, 56) f32 -> out: (32, 256, 56, 56) f32

v3 design:
- conv = 6 accumulating matmul passes per 8-row tile: 3 fp8-DoubleRow
  vertical tap pairs + 3 fp8-DoublePixel single taps, junk-free 4-level
  rhs APs ([[64,2],[64,8],[1,56]]) -> 448-wide PSUM tiles. DoublePixel
  halves the single-tap cost (2 pixels/cycle).
- conv2 fuses BN+residual into a PSUM preload (vector TS writes
  (x_act2 + b2)/S2 into PSUM; matmuls accumulate on top). Its post-op is
  one scalar-engine Identity (scale=S2) -> fp8; overflow saturates to
  +-inf which the host clip resolves. Host applies hardtanh.
- conv1-hi residual add writes fp8 directly (unclipped); host clips and
  adds the move1 even bias. conv1-lo stays f32 (ACT + add + clip
  bit-identical to the reference path: its values feed conv2's binarize,
  where one sign flip vs the f32 reference costs ~2.0 abs error).
- idle-hi quarter (input ch 192..255) never touches the device: its
  entire dataflow is x + move0_hi + move1_odd -> host computes it during
  reassembly.
- inputs: x_act + x_idle_lo as f32 (binarize-exactness), outputs ot/fo
  as fp8. Host reassembles the double channel shuffle.
"""

import numpy as np
import ml_dtypes

import concourse.bass as bass
import concourse.tile as tile
from concourse import bacc, mybir
from concourse import bass_utils

EPS = 1e-5
P = 128
H = W = 56
HW = H * W
WP = 64          # B row stride (fp8 bytes); DoubleRow pair stride = WP (16B mult)
RB = 59          # padded rows in B (1 + 56 + 1, plus a junk-read spare row)
TF = 8 * WP      # flat matmul free size per tile (512; 56-of-64 cols valid)
IMGS_PER_CORE = 4
NCORES = 8

F32 = mybir.dt.float32
F8 = mybir.dt.float8e4
ALU = mybir.AluOpType
ACTF = mybir.ActivationFunctionType
DR = mybir.MatmulPerfMode.DoubleRow
DP = mybir.MatmulPerfMode.DoublePixel

_CACHE = {}


def _flat(ap3):
    return ap3.rearrange("p r c -> p (r c)")


def _build():
    nc = bacc.Bacc("TRN2", target_bir_lowering=False, debug=False)

    xs_h = nc.dram_tensor("xs", [IMGS_PER_CORE, 192, H, W], F32, kind="ExternalInput")
    w1_h = nc.dram_tensor("w1m", [P, 9 * P], F8, kind="ExternalInput")
    w2_h = nc.dram_tensor("w2m", [P, 9 * P], F8, kind="ExternalInput")
    cst_h = nc.dram_tensor("cst", [P, 16], F32, kind="ExternalInput")
    ot_h = nc.dram_tensor("ot", [IMGS_PER_CORE, P, HW], F8, kind="ExternalOutput")
    fo_h = nc.dram_tensor("fo", [IMGS_PER_CORE, 64, HW], F8, kind="ExternalOutput")

    xs_ap = xs_h.ap()

    with tile.TileContext(nc) as tc:
        XA = [nc.alloc_sbuf_tensor(f"XA{i}", [P, H, W], F32).ap() for i in range(3)]
        V1 = [nc.alloc_sbuf_tensor(f"V1{i}", [P, H, W], F32).ap() for i in range(2)]
        B1 = [nc.alloc_sbuf_tensor(f"B1{i}", [P, RB, WP], F8).ap() for i in range(2)]
        B2 = [nc.alloc_sbuf_tensor(f"B2{i}", [P, RB, WP], F8).ap() for i in range(2)]
        FO = [nc.alloc_sbuf_tensor(f"FO{i}", [P, H, W], F8).ap() for i in range(2)]
        OT = [nc.alloc_sbuf_tensor(f"OT{i}", [P, H, W], F8).ap() for i in range(2)]
        VB = [nc.alloc_sbuf_tensor(f"VB{i}", [P, H, W], F32).ap() for i in range(2)]
        WS1 = nc.alloc_sbuf_tensor("WS1", [P, 9 * P], F8).ap()
        WS2 = nc.alloc_sbuf_tensor("WS2", [P, 9 * P], F8).ap()
        CST = nc.alloc_sbuf_tensor("CST", [P, 16], F32).ap()
        SCR = nc.alloc_sbuf_tensor("SCR", [P, 1], F32).ap()
        GARB = nc.alloc_sbuf_tensor("GARB", [P, 256], F8).ap()

        s1 = CST[:, 0:1]       # 2*inv1
        b1 = CST[:, 1:2]       # bn1 bias with c0 folded
        b2e = CST[:, 2:3]      # b2 (+ move0 bias on hi partitions)
        mv0e = CST[:, 6:7]     # 0 (lo) / move0_bias (hi)
        s2v = CST[:, 8:9]      # S2 = 2*inv2'

        # head loads: first x_act chunk gets queue priority
        nc.sync.dma_start(out=XA[0][:, 0:10], in_=xs_ap[0, 0:P, 0:10])
        nc.sync.dma_start(out=WS1, in_=w1_h.ap())
        nc.sync.dma_start(out=CST, in_=cst_h.ap())
        for r0, r1 in ((10, 28), (28, 42), (42, 56)):
            nc.sync.dma_start(out=XA[0][:, r0:r1], in_=xs_ap[0, 0:P, r0:r1])
        nc.sync.dma_start(out=WS2, in_=w2_h.ap())

        # pads stand for binarized zero-padding: conv1 runs in the {0,1}
        # u-domain (pad 0.5, c0 folded into b1), conv2 in the +-1 sign
        # domain (pad 0). Interiors are rewritten per image. B1[0]'s pads
        # go first (they gate image 0's binarize and the first matmul).
        def pad_b(_b, padv):
            _f = _flat(_b)
            nc.gpsimd.memset(_f[:, 0:WP], padv)
            nc.gpsimd.memset(_f[:, 57 * WP:RB * WP], padv)
            nc.gpsimd.memset(_b[:, 1:57, 0:1], padv)
            nc.gpsimd.memset(_b[:, 1:57, 57:64], padv)

        pad_b(B1[0], 0.5)

        # warm the scalar-engine activation table while DMAs run
        nc.vector.memset(SCR, 0.0)
        nc.scalar.activation(SCR, SCR, ACTF.Identity)

        pad_b(B1[1], 0.5)
        pad_b(B2[0], 0.0)
        pad_b(B2[1], 0.0)

        with (
            tc.tile_pool(name="psum1", bufs=2, space="PSUM") as psum1_pool,
            tc.tile_pool(name="psum2", bufs=2, space="PSUM") as psum2_pool,
            tc.tile_pool(name="stage", bufs=4) as stage_pool,
        ):

            def emit_conv_mms(ps, ws, bf, tp, npair, start, flat):
                """Conv matmuls for one tile pair: 3 vertical DoubleRow
                tap-pairs + 3 single taps, weight-outer across the pair so
                identical LDWEIGHTS are back to back. flat=True uses 512-wide
                rhs reads (fastest matmul shape; junk cols never consumed),
                flat=False uses junk-free 448 reads so an engine-preloaded
                PSUM accumulator lines up compactly (start=False)."""
                for g in range(3):  # pairs: taps (0,g)+(1,g), delta = WP
                    lhsT = bass.AP(tensor=ws.tensor, offset=ws.offset + 256 * g,
                                   ap=[list(ws.ap[0]), [P, 2], [1, P]])
                    for j in range(npair):
                        off = (8 * (2 * tp + j)) * WP + g
                        if flat:
                            rhs = bass.AP(tensor=bf.tensor, offset=bf.offset + off,
                                          ap=[list(bf.ap[0]), [WP, 2], [1, TF]])
                            out = ps[:, j, :]
                        else:
                            rhs = bass.AP(tensor=bf.tensor, offset=bf.offset + off,
                                          ap=[list(bf.ap[0]), [WP, 2], [WP, 8], [1, W]])
                            out = ps[:, j, 0:448]
                        nc.tensor.matmul(
                            out, lhsT=lhsT, rhs=rhs,
                            start=(start and g == 0), stop=False, perf_mode=DR,
                            skip_group_check=not start)
                for g in range(3):  # single taps (2,g)
                    lhsT = ws[:, 768 + P * g:768 + P * (g + 1)]
                    for j in range(npair):
                        off = (8 * (2 * tp + j) + 2) * WP + g
                        if flat:
                            rhs = bf[:, off:off + TF]
                            out = ps[:, j, :]
                        else:
                            rhs = bass.AP(tensor=bf.tensor, offset=bf.offset + off,
                                          ap=[list(bf.ap[0]), [WP, 8], [1, W]])
                            out = ps[:, j, 0:448]
                        nc.tensor.matmul(
                            out, lhsT=lhsT, rhs=rhs,
                            start=False, stop=(g == 2),
                            skip_group_check=not start)

            def ps_pair_view(ps, npair):
                # [P, npair, 448] compact view of a [P, 2, 512] PSUM tile
                # (valid cols only; junk cols keep garbage, never read)
                return bass.AP(tensor=ps.tensor, offset=ps.offset,
                               ap=[list(ps.ap[0]), [512, npair], [1, 448]])

            def ps_rows3(ps, npair, p0=0, p1=P, rstride=WP):
                # [p1-p0, npair, 8, 56] valid-column row view of a psum tile:
                # rstride=WP for flat-512 layouts (conv1), 56 for compact-448
                # layouts (conv2).
                tt = ps[p0:p1]
                return bass.AP(tensor=tt.tensor, offset=tt.offset,
                               ap=[list(tt.ap[0]), [512, npair], [rstride, 8], [1, W]])

            def st_rows3(t1, npair, p0=0, p1=P):
                tt = t1[p0:p1]
                return bass.AP(tensor=tt.tensor, offset=tt.offset,
                               ap=[list(tt.ap[0]), [448, npair], [56, 8], [1, W]])

            def hw_rows3(t, r0, npair, p0=0, p1=P):
                tt = _flat(t)[p0:p1]
                return bass.AP(tensor=tt.tensor, offset=tt.offset + r0 * W,
                               ap=[list(tt.ap[0]), [448, npair], [56, 8], [1, W]])

            def xa_load(n):
                nc.sync.dma_start(out=XA[n % 3], in_=xs_ap[n, 0:P])

            def u1(n, nchunks=2):
                """Binarize x_act into padded B1 ({0,1} u-domain). The first
                head chunk covers rows 0..18 so tile pair 0 gates on one op."""
                s = n % 2
                xa = XA[n % 3]
                bounds = ((0, 10), (10, 28), (28, 42), (42, 56)) if nchunks == 3 \
                    else ((0, 32), (32, 56))
                for r0, r1 in bounds:
                    nc.vector.tensor_scalar(
                        out=B1[s][:, 1 + r0:1 + r1, 1:57], in0=xa[:, r0:r1],
                        scalar1=0.0, scalar2=None, op0=ALU.is_ge)

            def u2_chunk(n, r0, r1):
                """Binarize x_act2 (= V1 lo | xil+mv0 hi) into padded B2
                (+-1 sign domain, on the scalar engine). fl(V1+mv0) then
                sign-extract matches the reference binarize bit-exactly."""
                nc.scalar.activation(
                    B2[n % 2][:, 1 + r0:1 + r1, 1:57],
                    V1[n % 2][:, r0:r1], ACTF.Sign, bias=mv0e)

            def xil_load(n):
                """Idle-lo load for image n (trails conv2(n-2), same slot)."""
                nc.sync.dma_start(out=V1[n % 2][64:128], in_=xs_ap[n, P:192])

            def conv1(n):
                s = n % 2
                xa, v1, fo = XA[n % 3], V1[s], FO[s]
                b1f = _flat(B1[s])
                for tp in range(4):  # tile pairs (0,1),(2,3),(4,5),(6,)
                    yield
                    npair = 2 if tp < 3 else 1
                    r0 = 16 * tp
                    if tp == 0 and n + 1 < IMGS_PER_CORE:
                        u1(n + 1)
                    ps = psum1_pool.tile([P, 2, 512], F32)
                    if n == 0 and tp == 0:
                        # ramp the PE clock on garbage data while the first
                        # input DMAs land; the real start=True matmuls below
                        # reset the accumulator
                        for _ in range(18):
                            nc.tensor.matmul(
                                ps[0:64, 0, 0:256], lhsT=GARB[:, 0:64],
                                rhs=GARB, start=True, stop=True,
                                skip_group_check=True)
                    emit_conv_mms(ps, WS1, b1f, tp, npair, start=True, flat=False)
                    t1 = stage_pool.tile([P, 2, 448], F32, tag="t1")
                    nc.scalar.activation(
                        st_rows3(t1, npair), ps_rows3(ps, npair, rstride=W),
                        ACTF.Identity, bias=b1, scale=s1)
                    # full-width f32 residual add (vector TT is the fastest TT)
                    nc.vector.tensor_tensor(
                        out=st_rows3(t1, npair),
                        in0=st_rows3(t1, npair),
                        in1=hw_rows3(xa, r0, npair), op=ALU.add)
                    # lo: exact f32 clip -> V1 (conv2 input)
                    nc.gpsimd.tensor_scalar(
                        out=hw_rows3(v1, r0, npair, 0, 64),
                        in0=st_rows3(t1, npair, 0, 64),
                        scalar1=1.0, scalar2=-1.0, op0=ALU.min, op1=ALU.max)
                    # hi: clipped residual sum to fp8 staging; host adds the
                    # move1 even bias
                    nc.gpsimd.tensor_scalar(
                        out=hw_rows3(fo, r0, npair, 64, 128),
                        in0=st_rows3(t1, npair, 64, 128),
                        scalar1=1.0, scalar2=-1.0, op0=ALU.min, op1=ALU.max)
                    if tp >= 1 and n > 0:
                        u2_chunk(n, 16 * (tp - 1), 16 * tp)
                yield
                nc.sync.dma_start(out=fo_h.ap()[n], in_=_flat(fo)[64:128])
                if n > 0:
                    u2_chunk(n, 48, 56)
                else:
                    # image 0: binarize after all ACT1s so the scalar queue
                    # never blocks conv1(1)'s PSUM recycling
                    for c0, c1 in ((0, 16), (16, 32), (32, 48), (48, 56)):
                        u2_chunk(n, c0, c1)

            def conv2(n):
                s = n % 2
                v1, ot, vb = V1[s], OT[s], VB[s]
                b2f = _flat(B2[s])
                final = n == IMGS_PER_CORE - 1
                for tp in range(4):
                    yield
                    npair = 2 if tp < 3 else 1
                    nr = 8 * npair
                    r0 = 16 * tp
                    ps = psum2_pool.tile([P, 2, 512], F32)
                    emit_conv_mms(ps, WS2, b2f, tp, npair, start=True, flat=False)
                    # residual + BN bias staging: V1b = x_act2 + b2ext
                    nc.scalar.activation(
                        hw_rows3(vb, r0, npair), hw_rows3(v1, r0, npair),
                        ACTF.Identity, bias=b2e)
                    # fused post: S2*conv + (x_act2 + b2) -> fp8 (overflow
                    # saturates to +-inf); host applies the hardtanh clip
                    nc.vector.scalar_tensor_tensor(
                        out=hw_rows3(ot, r0, npair),
                        in0=ps_rows3(ps, npair, rstride=W),
                        scalar=s2v, in1=hw_rows3(vb, r0, npair),
                        op0=ALU.mult, op1=ALU.add)
                    if final:
                        # tail: flush each tile pair as soon as it's ready
                        nc.sync.dma_start(
                            out=ot_h.ap()[n][:, r0 * W:(r0 + nr) * W],
                            in_=_flat(ot)[:, r0 * W:(r0 + nr) * W])
                yield
                if not final:
                    nc.sync.dma_start(out=ot_h.ap()[n], in_=_flat(ot))

            def run_all(gen):
                if gen is not None:
                    for _ in gen:
                        pass

            # software pipeline across images: conv1(n+1) is emitted before
            # conv2(n) so the PE never stalls on the u2(n) dependency chain.
            xa_load(1)
            xa_load(2)
            xil_load(0)
            xil_load(1)
            u1(0, nchunks=3)
            with nc.named_scope("c1_0"):
                run_all(conv1(0))
            xa_load(3)
            g_prev = conv1(1)
            for n in range(IMGS_PER_CORE):
                if n >= 1 and n + 1 < IMGS_PER_CORE:
                    xil_load(n + 1)
                g_c2 = conv2(n)
                if n + 2 < IMGS_PER_CORE:
                    g_next = conv1(n + 2)
                else:
                    g_next = None
                if g_prev is not None:
                    with nc.named_scope(f"c1_{n + 1}"):
                        run_all(g_prev)
                with nc.named_scope(f"c2_{n}"):
                    run_all(g_c2)
                g_prev = g_next

    nc.compile()
    return nc


def _host_prep(w1, w2, bn1_gamma, bn1_beta, bn1_mean, bn1_var,
               bn2_gamma, bn2_beta, bn2_mean, bn2_var, move0_bias, move1_bias):
    f8 = np.float64
    bw1 = np.where(w1 >= 0, 1.0, -1.0).astype(f8)   # [co, ci, 3, 3]
    bw2 = np.where(w2 >= 0, 1.0, -1.0).astype(f8)

    def wlayout(bw):
        # [ci, 1152]: 3 DoubleRow groups (taps (0,g),(1,g)) then 3 singles
        # (taps (2,g)); within a group the two taps' [ci, co] blocks are
        # adjacent (matching the lhsT [K, 2, M] access pattern).
        m = np.zeros((P, 9 * P), np.float64)
        t = bw.transpose(2, 3, 1, 0)  # [ky, kx, ci, co]
        for g in range(3):
            m[:, 256 * g:256 * g + 128] = t[0, g]
            m[:, 256 * g + 128:256 * g + 256] = t[1, g]
            m[:, 768 + 128 * g:768 + 128 * (g + 1)] = t[2, g]
        return np.ascontiguousarray(m).astype(ml_dtypes.float8_e4m3)

    w1m = wlayout(bw1)

    # conv2 channel permutation (both in and out sides)
    pidx = np.arange(P)
    chan = np.where(pidx < 64, 2 * pidx, 2 * (pidx - 64) + 1)
    bw2p = bw2[np.ix_(chan, chan)]
    w2m = wlayout(bw2p)

    # u-domain: conv_sign = 2*conv_u - c0, c0 = sum of signed weights
    inv1 = bn1_gamma.astype(f8) / np.sqrt(bn1_var.astype(f8) + EPS)
    c0_1 = bw1.sum(axis=(1, 2, 3))
    s1 = 2.0 * inv1
    b1 = bn1_beta.astype(f8) - bn1_mean.astype(f8) * inv1 - inv1 * c0_1

    # conv2 runs in the +-1 sign domain (0-pads): no c0 fold, no 2x scale
    inv2 = (bn2_gamma.astype(f8) / np.sqrt(bn2_var.astype(f8) + EPS))[chan]
    S2 = inv2
    b2 = bn2_beta.astype(f8)[chan] - bn2_mean.astype(f8)[chan] * inv2

    mv0ext = np.concatenate([np.zeros(64), move0_bias.astype(f8)[:64]])

    cst = np.zeros((P, 16), np.float64)
    cst[:, 0] = s1
    cst[:, 1] = b1
    cst[:, 2] = mv0ext + b2
    cst[:, 6] = mv0ext
    cst[:, 8] = S2

    i = np.arange(64)
    host = {
        "mv1e": move1_bias.astype(np.float32)[2 * i],   # [64]
        "mv0h": move0_bias.astype(np.float32)[64 + i],  # [64]
        "mv1o": move1_bias.astype(np.float32)[2 * i + 1],
    }
    return w1m, w2m, cst.astype(np.float32), host


def kernel(x, w1, w2, bn1_gamma, bn1_beta, bn1_mean, bn1_var,
           bn2_gamma, bn2_beta, bn2_mean, bn2_var, move0_bias, move1_bias,
           _trace=False):
    x = np.asarray(x, np.float32)
    args = [np.asarray(a, np.float32) for a in (
        w1, w2, bn1_gamma, bn1_beta, bn1_mean, bn1_var,
        bn2_gamma, bn2_beta, bn2_mean, bn2_var, move0_bias, move1_bias)]
    w1m, w2m, cst, host = _host_prep(*args)

    if "nc" not in _CACHE:
        _CACHE["nc"] = _build()
    nc = _CACHE["nc"]

    n_img = IMGS_PER_CORE
    in_maps = [
        {"xs": np.ascontiguousarray(x[n_img * c:n_img * (c + 1), 0:192]),
         "w1m": w1m, "w2m": w2m, "cst": cst}
        for c in range(NCORES)
    ]
    kw = {}
    if _trace:
        kw = dict(trace=True, trace_kwargs={"title": "basicblock"})
    res = bass_utils.run_bass_kernel_spmd(nc, in_maps, core_ids=list(range(NCORES)), **kw)

    ot = np.concatenate([res.results[c]["ot"] for c in range(NCORES)], axis=0)
    fo = np.concatenate([res.results[c]["fo"] for c in range(NCORES)], axis=0)
    if _trace:
        _CACHE["last_results"] = res

    N = x.shape[0]
    out = np.empty((N, 2 * P, H, W), np.float32)
    ht2 = np.clip(ot.astype(np.float32), -1.0, 1.0)
    out[:, 0::4] = ht2[:, 0:64].reshape(N, 64, H, W)
    out[:, 2::4] = ht2[:, 64:128].reshape(N, 64, H, W)
    out[:, 1::4] = (np.clip(fo.astype(np.float32), -1.0, 1.0) +
                    host["mv1e"][None, :, None]).reshape(N, 64, H, W)
    # idle-hi path entirely on host: x + move0_hi + move1_odd
    xh = x[:, 192:256] + host["mv0h"][None, :, None, None]
    out[:, 3::4] = xh + host["mv1o"][None, :, None, None]
    return out


# revision 39
# speedup vs baseline: 1.0418x; 1.0418x over previous
"""Trainium2 Bass kernel for a binarized BasicBlock (2x bconv3x3 + BN +
residual hardtanh + channel shuffle), data-parallel over batch on 8 cores.

Self-contained: hardcodes shapes from the problem spec.
  x: (32, 256, 56, 56) f32 -> out: (32, 256, 56, 56) f32

v3 design:
- conv = 6 accumulating matmul passes per 8-row tile: 3 fp8-DoubleRow
  vertical tap pairs + 3 fp8-DoublePixel single taps, junk-free 4-level
  rhs APs ([[64,2],[64,8],[1,56]]) -> 448-wide PSUM tiles. DoublePixel
  halves the single-tap cost (2 pixels/cycle).
- conv2 fuses BN+residual into a PSUM preload (vector TS writes
  (x_act2 + b2)/S2 into PSUM; matmuls accumulate on top). Its post-op is
  one scalar-engine Identity (scale=S2) -> fp8; overflow saturates to
  +-inf which the host clip resolves. Host applies hardtanh.
- conv1-hi residual add writes fp8 directly (unclipped); host clips and
  adds the move1 even bias. conv1-lo stays f32 (ACT + add + clip
  bit-identical to the reference path: its values feed conv2's binarize,
  where one sign flip vs the f32 reference costs ~2.0 abs error).
- idle-hi quarter (input ch 192..255) never touches the device: its
  entire dataflow is x + move0_hi + move1_odd -> host computes it during
  reassembly.
- inputs: x_act + x_idle_lo as f32 (binarize-exactness), outputs ot/fo
  as fp8. Host reassembles the double channel shuffle.
"""

import numpy as np
import ml_dtypes

import concourse.bass as bass
import concourse.tile as tile
from concourse import bacc, mybir
from concourse import bass_utils

EPS = 1e-5
P = 128
H = W = 56
HW = H * W
WP = 64          # B row stride (fp8 bytes); DoubleRow pair stride = WP (16B mult)
RB = 59          # padded rows in B (1 + 56 + 1, plus a junk-read spare row)
TF = 8 * WP      # flat matmul free size per tile (512; 56-of-64 cols valid)
IMGS_PER_CORE = 4
NCORES = 8

F32 = mybir.dt.float32
F8 = mybir.dt.float8e4
ALU = mybir.AluOpType
ACTF = mybir.ActivationFunctionType
DR = mybir.MatmulPerfMode.DoubleRow
DP = mybir.MatmulPerfMode.DoublePixel

_CACHE = {}


def _flat(ap3):
    return ap3.rearrange("p r c -> p (r c)")


def _build():
    nc = bacc.Bacc("TRN2", target_bir_lowering=False, debug=False)

    xs_h = nc.dram_tensor("xs", [IMGS_PER_CORE, 192, H, W], F32, kind="ExternalInput")
    w1_h = nc.dram_tensor("w1m", [P, 9 * P], F8, kind="ExternalInput")
    w2_h = nc.dram_tensor("w2m", [P, 9 * P], F8, kind="ExternalInput")
    cst_h = nc.dram_tensor("cst", [P, 16], F32, kind="ExternalInput")
    ot_h = nc.dram_tensor("ot", [IMGS_PER_CORE, P, HW], F8, kind="ExternalOutput")
    fo_h = nc.dram_tensor("fo", [IMGS_PER_CORE, 64, HW], F8, kind="ExternalOutput")

    xs_ap = xs_h.ap()

    with tile.TileContext(nc) as tc:
        XA = [nc.alloc_sbuf_tensor(f"XA{i}", [P, H, W], F32).ap() for i in range(3)]
        V1 = [nc.alloc_sbuf_tensor(f"V1{i}", [P, H, W], F32).ap() for i in range(2)]
        B1 = [nc.alloc_sbuf_tensor(f"B1{i}", [P, RB, WP], F8).ap() for i in range(2)]
        B2 = [nc.alloc_sbuf_tensor(f"B2{i}", [P, RB, WP], F8).ap() for i in range(2)]
        FO = [nc.alloc_sbuf_tensor(f"FO{i}", [P, H, W], F8).ap() for i in range(2)]
        OT = [nc.alloc_sbuf_tensor(f"OT{i}", [P, H, W], F8).ap() for i in range(2)]
        VB = [nc.alloc_sbuf_tensor(f"VB{i}", [P, H, W], F32).ap() for i in range(2)]
        WS1 = nc.alloc_sbuf_tensor("WS1", [P, 9 * P], F8).ap()
        WS2 = nc.alloc_sbuf_tensor("WS2", [P, 9 * P], F8).ap()
        CST = nc.alloc_sbuf_tensor("CST", [P, 16], F32).ap()
        SCR = nc.alloc_sbuf_tensor("SCR", [P, 1], F32).ap()
        GARB = nc.alloc_sbuf_tensor("GARB", [P, 256], F8).ap()

        s1 = CST[:, 0:1]       # 2*inv1
        b1 = CST[:, 1:2]       # bn1 bias with c0 folded
        b2e = CST[:, 2:3]      # b2 (+ move0 bias on hi partitions)
        mv0e = CST[:, 6:7]     # 0 (lo) / move0_bias (hi)
        s2v = CST[:, 8:9]      # S2 = 2*inv2'

        # head loads: first x_act chunk gets queue priority
        nc.sync.dma_start(out=XA[0][:, 0:10], in_=xs_ap[0, 0:P, 0:10])
        nc.sync.dma_start(out=WS1, in_=w1_h.ap())
        nc.sync.dma_start(out=CST, in_=cst_h.ap())
        for r0, r1 in ((10, 28), (28, 56)):
            nc.sync.dma_start(out=XA[0][:, r0:r1], in_=xs_ap[0, 0:P, r0:r1])
        nc.sync.dma_start(out=WS2, in_=w2_h.ap())

        # pads stand for binarized zero-padding: conv1 runs in the {0,1}
        # u-domain (pad 0.5, c0 folded into b1), conv2 in the +-1 sign
        # domain (pad 0). Interiors are rewritten per image. B1[0]'s pads
        # go first (they gate image 0's binarize and the first matmul).
        def pad_b(_b, padv):
            _f = _flat(_b)
            nc.gpsimd.memset(_f[:, 0:WP], padv)
            nc.gpsimd.memset(_f[:, 57 * WP:RB * WP], padv)
            nc.gpsimd.memset(_b[:, 1:57, 0:1], padv)
            nc.gpsimd.memset(_b[:, 1:57, 57:64], padv)

        pad_b(B1[0], 0.5)

        # warm the scalar-engine activation table while DMAs run
        nc.vector.memset(SCR, 0.0)
        nc.scalar.activation(SCR, SCR, ACTF.Identity)

        pad_b(B1[1], 0.5)
        pad_b(B2[0], 0.0)
        pad_b(B2[1], 0.0)

        with (
            tc.tile_pool(name="psum1", bufs=2, space="PSUM") as psum1_pool,
            tc.tile_pool(name="psum2", bufs=2, space="PSUM") as psum2_pool,
            tc.tile_pool(name="stage", bufs=4) as stage_pool,
        ):

            def emit_conv_mms(ps, ws, bf, tp, npair, start, flat):
                """Conv matmuls for one tile pair: 3 vertical DoubleRow
                tap-pairs + 3 single taps, weight-outer across the pair so
                identical LDWEIGHTS are back to back. flat=True uses 512-wide
                rhs reads (fastest matmul shape; junk cols never consumed),
                flat=False uses junk-free 448 reads so an engine-preloaded
                PSUM accumulator lines up compactly (start=False)."""
                for g in range(3):  # pairs: taps (0,g)+(1,g), delta = WP
                    lhsT = bass.AP(tensor=ws.tensor, offset=ws.offset + 256 * g,
                                   ap=[list(ws.ap[0]), [P, 2], [1, P]])
                    for j in range(npair):
                        off = (8 * (2 * tp + j)) * WP + g
                        if flat:
                            rhs = bass.AP(tensor=bf.tensor, offset=bf.offset + off,
                                          ap=[list(bf.ap[0]), [WP, 2], [1, TF]])
                            out = ps[:, j, :]
                        else:
                            rhs = bass.AP(tensor=bf.tensor, offset=bf.offset + off,
                                          ap=[list(bf.ap[0]), [WP, 2], [WP, 8], [1, W]])
                            out = ps[:, j, 0:448]
                        nc.tensor.matmul(
                            out, lhsT=lhsT, rhs=rhs,
                            start=(start and g == 0), stop=False, perf_mode=DR,
                            skip_group_check=not start)
                for g in range(3):  # single taps (2,g)
                    lhsT = ws[:, 768 + P * g:768 + P * (g + 1)]
                    for j in range(npair):
                        off = (8 * (2 * tp + j) + 2) * WP + g
                        if flat:
                            rhs = bf[:, off:off + TF]
                            out = ps[:, j, :]
                        else:
                            rhs = bass.AP(tensor=bf.tensor, offset=bf.offset + off,
                                          ap=[list(bf.ap[0]), [WP, 8], [1, W]])
                            out = ps[:, j, 0:448]
                        nc.tensor.matmul(
                            out, lhsT=lhsT, rhs=rhs,
                            start=False, stop=(g == 2),
                            skip_group_check=not start)

            def ps_pair_view(ps, npair):
                # [P, npair, 448] compact view of a [P, 2, 512] PSUM tile
                # (valid cols only; junk cols keep garbage, never read)
                return bass.AP(tensor=ps.tensor, offset=ps.offset,
                               ap=[list(ps.ap[0]), [512, npair], [1, 448]])

            def ps_rows3(ps, npair, p0=0, p1=P, rstride=WP):
                # [p1-p0, npair, 8, 56] valid-column row view of a psum tile:
                # rstride=WP for flat-512 layouts (conv1), 56 for compact-448
                # layouts (conv2).
                tt = ps[p0:p1]
                return bass.AP(tensor=tt.tensor, offset=tt.offset,
                               ap=[list(tt.ap[0]), [512, npair], [rstride, 8], [1, W]])

            def st_rows3(t1, npair, p0=0, p1=P):
                tt = t1[p0:p1]
                return bass.AP(tensor=tt.tensor, offset=tt.offset,
                               ap=[list(tt.ap[0]), [448, npair], [56, 8], [1, W]])

            def hw_rows3(t, r0, npair, p0=0, p1=P):
                tt = _flat(t)[p0:p1]
                return bass.AP(tensor=tt.tensor, offset=tt.offset + r0 * W,
                               ap=[list(tt.ap[0]), [448, npair], [56, 8], [1, W]])

            def xa_load(n):
                nc.sync.dma_start(out=XA[n % 3], in_=xs_ap[n, 0:P])

            def u1(n, nchunks=2):
                """Binarize x_act into padded B1 ({0,1} u-domain). The first
                head chunk covers rows 0..18 so tile pair 0 gates on one op."""
                s = n % 2
                xa = XA[n % 3]
                bounds = ((0, 10), (10, 28), (28, 56)) if nchunks == 3 \
                    else ((0, 32), (32, 56))
                for r0, r1 in bounds:
                    nc.vector.tensor_scalar(
                        out=B1[s][:, 1 + r0:1 + r1, 1:57], in0=xa[:, r0:r1],
                        scalar1=0.0, scalar2=None, op0=ALU.is_ge)

            def u2_chunk(n, r0, r1):
                """Binarize x_act2 (= V1 lo | xil+mv0 hi) into padded B2
                (+-1 sign domain, on the scalar engine). fl(V1+mv0) then
                sign-extract matches the reference binarize bit-exactly."""
                nc.scalar.activation(
                    B2[n % 2][:, 1 + r0:1 + r1, 1:57],
                    V1[n % 2][:, r0:r1], ACTF.Sign, bias=mv0e)

            def xil_load(n):
                """Idle-lo load for image n (trails conv2(n-2), same slot)."""
                nc.sync.dma_start(out=V1[n % 2][64:128], in_=xs_ap[n, P:192])

            def conv1(n):
                s = n % 2
                xa, v1, fo = XA[n % 3], V1[s], FO[s]
                b1f = _flat(B1[s])
                for tp in range(4):  # tile pairs (0,1),(2,3),(4,5),(6,)
                    yield
                    npair = 2 if tp < 3 else 1
                    r0 = 16 * tp
                    if tp == 0 and n + 1 < IMGS_PER_CORE:
                        u1(n + 1)
                    ps = psum1_pool.tile([P, 2, 512], F32)
                    if n == 0 and tp == 0:
                        # ramp the PE clock on garbage data while the first
                        # input DMAs land; the real start=True matmuls below
                        # reset the accumulator
                        for _ in range(18):
                            nc.tensor.matmul(
                                ps[0:64, 0, 0:256], lhsT=GARB[:, 0:64],
                                rhs=GARB, start=True, stop=True,
                                skip_group_check=True)
                    emit_conv_mms(ps, WS1, b1f, tp, npair, start=True, flat=False)
                    t1 = stage_pool.tile([P, 2, 448], F32, tag="t1")
                    nc.scalar.activation(
                        st_rows3(t1, npair), ps_rows3(ps, npair, rstride=W),
                        ACTF.Identity, bias=b1, scale=s1)
                    # full-width f32 residual add (vector TT is the fastest TT)
                    nc.vector.tensor_tensor(
                        out=st_rows3(t1, npair),
                        in0=st_rows3(t1, npair),
                        in1=hw_rows3(xa, r0, npair), op=ALU.add)
                    # lo: exact f32 clip -> V1 (conv2 input)
                    nc.gpsimd.tensor_scalar(
                        out=hw_rows3(v1, r0, npair, 0, 64),
                        in0=st_rows3(t1, npair, 0, 64),
                        scalar1=1.0, scalar2=-1.0, op0=ALU.min, op1=ALU.max)
                    # hi: clipped residual sum to fp8 staging; host adds the
                    # move1 even bias
                    nc.gpsimd.tensor_scalar(
                        out=hw_rows3(fo, r0, npair, 64, 128),
                        in0=st_rows3(t1, npair, 64, 128),
                        scalar1=1.0, scalar2=-1.0, op0=ALU.min, op1=ALU.max)
                    if tp >= 1 and n > 0:
                        u2_chunk(n, 16 * (tp - 1), 16 * tp)
                yield
                nc.sync.dma_start(out=fo_h.ap()[n], in_=_flat(fo)[64:128])
                if n > 0:
                    u2_chunk(n, 48, 56)
                else:
                    # image 0: binarize after all ACT1s so the scalar queue
                    # never blocks conv1(1)'s PSUM recycling
                    for c0, c1 in ((0, 16), (16, 32), (32, 48), (48, 56)):
                        u2_chunk(n, c0, c1)

            def conv2(n):
                s = n % 2
                v1, ot, vb = V1[s], OT[s], VB[s]
                b2f = _flat(B2[s])
                final = n == IMGS_PER_CORE - 1
                for tp in range(4):
                    yield
                    npair = 2 if tp < 3 else 1
                    nr = 8 * npair
                    r0 = 16 * tp
                    ps = psum2_pool.tile([P, 2, 512], F32)
                    emit_conv_mms(ps, WS2, b2f, tp, npair, start=True, flat=False)
                    # residual + BN bias staging: V1b = x_act2 + b2ext
                    nc.scalar.activation(
                        hw_rows3(vb, r0, npair), hw_rows3(v1, r0, npair),
                        ACTF.Identity, bias=b2e)
                    # fused post: S2*conv + (x_act2 + b2) -> fp8 (overflow
                    # saturates to +-inf); host applies the hardtanh clip
                    nc.vector.scalar_tensor_tensor(
                        out=hw_rows3(ot, r0, npair),
                        in0=ps_rows3(ps, npair, rstride=W),
                        scalar=s2v, in1=hw_rows3(vb, r0, npair),
                        op0=ALU.mult, op1=ALU.add)
                    if final:
                        # tail: flush each tile pair as soon as it's ready
                        nc.sync.dma_start(
                            out=ot_h.ap()[n][:, r0 * W:(r0 + nr) * W],
                            in_=_flat(ot)[:, r0 * W:(r0 + nr) * W])
                yield
                if not final:
                    nc.sync.dma_start(out=ot_h.ap()[n], in_=_flat(ot))

            def run_all(gen):
                if gen is not None:
                    for _ in gen:
                        pass

            # software pipeline across images: conv1(n+1) is emitted before
            # conv2(n) so the PE never stalls on the u2(n) dependency chain.
            xa_load(1)
            xa_load(2)
            xil_load(0)
            xil_load(1)
            u1(0, nchunks=3)
            with nc.named_scope("c1_0"):
                run_all(conv1(0))
            xa_load(3)
            g_prev = conv1(1)
            for n in range(IMGS_PER_CORE):
                if n >= 1 and n + 1 < IMGS_PER_CORE:
                    xil_load(n + 1)
                g_c2 = conv2(n)
                if n + 2 < IMGS_PER_CORE:
                    g_next = conv1(n + 2)
                else:
                    g_next = None
                if g_prev is not None:
                    with nc.named_scope(f"c1_{n + 1}"):
                        run_all(g_prev)
                with nc.named_scope(f"c2_{n}"):
                    run_all(g_c2)
                g_prev = g_next

    nc.compile()
    return nc


def _host_prep(w1, w2, bn1_gamma, bn1_beta, bn1_mean, bn1_var,
               bn2_gamma, bn2_beta, bn2_mean, bn2_var, move0_bias, move1_bias):
    f8 = np.float64
    bw1 = np.where(w1 >= 0, 1.0, -1.0).astype(f8)   # [co, ci, 3, 3]
    bw2 = np.where(w2 >= 0, 1.0, -1.0).astype(f8)

    def wlayout(bw):
        # [ci, 1152]: 3 DoubleRow groups (taps (0,g),(1,g)) then 3 singles
        # (taps (2,g)); within a group the two taps' [ci, co] blocks are
        # adjacent (matching the lhsT [K, 2, M] access pattern).
        m = np.zeros((P, 9 * P), np.float64)
        t = bw.transpose(2, 3, 1, 0)  # [ky, kx, ci, co]
        for g in range(3):
            m[:, 256 * g:256 * g + 128] = t[0, g]
            m[:, 256 * g + 128:256 * g + 256] = t[1, g]
            m[:, 768 + 128 * g:768 + 128 * (g + 1)] = t[2, g]
        return np.ascontiguousarray(m).astype(ml_dtypes.float8_e4m3)

    w1m = wlayout(bw1)

    # conv2 channel permutation (both in and out sides)
    pidx = np.arange(P)
    chan = np.where(pidx < 64, 2 * pidx, 2 * (pidx - 64) + 1)
    bw2p = bw2[np.ix_(chan, chan)]
    w2m = wlayout(bw2p)

    # u-domain: conv_sign = 2*conv_u - c0, c0 = sum of signed weights
    inv1 = bn1_gamma.astype(f8) / np.sqrt(bn1_var.astype(f8) + EPS)
    c0_1 = bw1.sum(axis=(1, 2, 3))
    s1 = 2.0 * inv1
    b1 = bn1_beta.astype(f8) - bn1_mean.astype(f8) * inv1 - inv1 * c0_1

    # conv2 runs in the +-1 sign domain (0-pads): no c0 fold, no 2x scale
    inv2 = (bn2_gamma.astype(f8) / np.sqrt(bn2_var.astype(f8) + EPS))[chan]
    S2 = inv2
    b2 = bn2_beta.astype(f8)[chan] - bn2_mean.astype(f8)[chan] * inv2

    mv0ext = np.concatenate([np.zeros(64), move0_bias.astype(f8)[:64]])

    cst = np.zeros((P, 16), np.float64)
    cst[:, 0] = s1
    cst[:, 1] = b1
    cst[:, 2] = mv0ext + b2
    cst[:, 6] = mv0ext
    cst[:, 8] = S2

    i = np.arange(64)
    host = {
        "mv1e": move1_bias.astype(np.float32)[2 * i],   # [64]
        "mv0h": move0_bias.astype(np.float32)[64 + i],  # [64]
        "mv1o": move1_bias.astype(np.float32)[2 * i + 1],
    }
    return w1m, w2m, cst.astype(np.float32), host


def kernel(x, w1, w2, bn1_gamma, bn1_beta, bn1_mean, bn1_var,
           bn2_gamma, bn2_beta, bn2_mean, bn2_var, move0_bias, move1_bias,
           _trace=False):
    x = np.asarray(x, np.float32)
    args = [np.asarray(a, np.float32) for a in (
        w1, w2, bn1_gamma, bn1_beta, bn1_mean, bn1_var,
        bn2_gamma, bn2_beta, bn2_mean, bn2_var, move0_bias, move1_bias)]
    w1m, w2m, cst, host = _host_prep(*args)

    if "nc" not in _CACHE:
        _CACHE["nc"] = _build()
    nc = _CACHE["nc"]

    n_img = IMGS_PER_CORE
    in_maps = [
        {"xs": np.ascontiguousarray(x[n_img * c:n_img * (c + 1), 0:192]),
         "w1m": w1m, "w2m": w2m, "cst": cst}
        for c in range(NCORES)
    ]
    kw = {}
    if _trace:
        kw = dict(trace=True, trace_kwargs={"title": "basicblock"})
    res = bass_utils.run_bass_kernel_spmd(nc, in_maps, core_ids=list(range(NCORES)), **kw)

    ot = np.concatenate([res.results[c]["ot"] for c in range(NCORES)], axis=0)
    fo = np.concatenate([res.results[c]["fo"] for c in range(NCORES)], axis=0)
    if _trace:
        _CACHE["last_results"] = res

    N = x.shape[0]
    out = np.empty((N, 2 * P, H, W), np.float32)
    ht2 = np.clip(ot.astype(np.float32), -1.0, 1.0)
    out[:, 0::4] = ht2[:, 0:64].reshape(N, 64, H, W)
    out[:, 2::4] = ht2[:, 64:128].reshape(N, 64, H, W)
    out[:, 1::4] = (np.clip(fo.astype(np.float32), -1.0, 1.0) +
                    host["mv1e"][None, :, None]).reshape(N, 64, H, W)
    # idle-hi path entirely on host: x + move0_hi + move1_odd
    xh = x[:, 192:256] + host["mv0h"][None, :, None, None]
    out[:, 3::4] = xh + host["mv1o"][None, :, None, None]
    return out
